# revision 38
# baseline (speedup 1.0000x reference)
"""AnchorStripeAttention Trainium2 kernel (8 NeuronCores, data-parallel over windows).

Host: window-partition + per-head l2norm + logit-scale fold. CPB bias folded
into the QK matmuls as extra contraction rows, rank-32 factorized for BOTH
stages (bias centered along the softmax axis first; the dropped mean is
softmax-invariant). Heads are paired (j, j+3) block-diagonally so every QK
matmul is a full [K=128, M=128] stationary.

Device (per window): 6 stage-1 QK matmuls (K=128: k 2x32 + W1 2x32, N=128:
anc|V1 block-diag), 3 stage-2 QK matmuls (K=128, N=256), ONE merged exp over
the [128,1536] score tile on ACT, 12 AV matmuls (K=128, N=66) with
ones-column denominators, normalize on DVE.

Scheduling notes:
- Both stages' scores live in ONE [128,1536] PSUM tile (3 banks, 2-slot
  rotation + 2 xo banks = 8 banks exactly). One exp per window saves ~240ns
  of ACT overhead/window; ACT (~1.54us/window) is the steady-state pacer.
- DMA: each path (sync HWDGE / scalar HWDGE / pool SWDGE) moves ~2.6 B/ns
  per partition-row and serializes its transfers, so path time =
  sum(row_bytes)/2.6. Per 4-window group: pool = kin [64-row] + anc f0/f1
  strided [32-row] + vin [128-row] (~4.8us), sync = qin [64-row] + out
  halves [128-row] (~4.7us) vs a ~6.3us group budget. Constants stream on
  scalar, interleaved per parity so group 0 starts after one parity's worth.
- 3 parity sets of kt/an/qa/va tiles (const rows live below the DMA rows);
  kin/qin prefetch 3 groups ahead, vin 2 groups, out flushed in half-groups.
- The rank-32 stage-1 bias costs ~5e-3 extra rel err (1.4e-2 total vs 2e-2
  budget) and makes kin a 64-row DMA (2x faster path class), kills the I64
  identity columns, and shrinks the replicated constants ~40%.
"""

import math
import sys

import numpy as np

if "/opt/trn_rl_repo" not in sys.path:
    sys.path.insert(0, "/opt/trn_rl_repo")

import concourse.bass as bass  # noqa: E402
import concourse.bacc as bacc  # noqa: E402
import concourse.tile as tile  # noqa: E402
from concourse import mybir  # noqa: E402
from concourse.bass_utils import run_bass_kernel_spmd  # noqa: E402

import ml_dtypes  # noqa: E402

BF16 = np.dtype(ml_dtypes.bfloat16)

NUM_HEADS = 6
DIM = 192
HD = 32
STRIPE = 16
ANCH = 8
B = 2
HS = 256
N1 = STRIPE * STRIPE  # 256 window tokens
N2 = ANCH * ANCH      # 64 anchor tokens
NWIN = 512
NCORES = 8
WPC = NWIN // NCORES  # 64 windows per core
LOGIT_MAX = math.log(1.0 / 0.01)
RANK2 = 32            # stage-2 bias factor rank

# kt tile [128, 768]: block (j,t) at cols 128*(2j+t): rows 0:32 k_j^T,
# rows 32:64 k_{j+3}^T (per-window DMA); rows 64:96 W1_j, 96:128 W1_{j+3}
# (const rank-32 stage-1 bias factors over tokens).
KT_W = 768
# an tile [128, 384]: block j at cols 128j; cols 0:64 (f0): rows 0:32
# anc_j^T, rows 64:96 V1_j; cols 64:128 (f1): rows 32:64 anc_{j+3}^T,
# rows 96:128 V1_{j+3}; other cells zero (block-diag head pairing).
AN_W = 384
# qa tile [128, 1152]: rows 0-63 per-window; cols 256j hold qT stacked f0/f1;
# cols 768+128j hold anc block-diag. rows 64-127: u2 stacked + v2 bd (const).
QA_W = 1152
A2_OFF = 768
# va tile [128, 396]: block (t,j) at 198t+66j: [f0 v(32) | f1 v(32) | one | 0]
VA_W = 396
S_W = 1536            # psum: stage1 (j,t,f) 64-blocks at 256j+128t+64f; stage2 at 768+256j
S2_OFF = 768          # stage-2 score offset within the merged [128,1536] S tile

_CACHED = {}


def _build_nc():
    BF = mybir.dt.bfloat16
    F32 = mybir.dt.float32
    EXP = mybir.ActivationFunctionType.Exp

    GRP = 4                   # windows per DMA group
    NG = WPC // GRP

    nc = bacc.Bacc(None)
    kin_d = nc.dram_tensor("kin", [NG, 64, GRP * KT_W], BF, kind="ExternalInput")
    ain_d = nc.dram_tensor("ain", [NG, 2, 32, GRP * 192], BF, kind="ExternalInput")
    qin_d = nc.dram_tensor("qin", [NG, 64, GRP * QA_W], BF, kind="ExternalInput")
    vin_d = nc.dram_tensor("vin", [NG, 128, GRP * VA_W], BF, kind="ExternalInput")
    kc_d = nc.dram_tensor("kc", [64, GRP * KT_W], BF, kind="ExternalInput")
    vc_d = nc.dram_tensor("vc", [64, GRP * AN_W], BF, kind="ExternalInput")
    qc_d = nc.dram_tensor("qc", [64, GRP * QA_W], BF, kind="ExternalInput")
    out_d = nc.dram_tensor("out", [NG, 128, GRP, 12, 32], BF, kind="ExternalOutput")

    with tile.TileContext(nc) as tc:
        with (
            tc.tile_pool(name="const", bufs=1) as constp,
            tc.tile_pool(name="esp", bufs=5) as esp,
            tc.tile_pool(name="rdp", bufs=2) as rdp,
            tc.tile_pool(name="rop", bufs=2) as rop,
            tc.tile_pool(name="ps_s", bufs=2, space="PSUM") as ps_s,
            tc.tile_pool(name="ps_xo", bufs=2, space="PSUM") as ps_xo,
        ):
            kts, ans, qas, vas, ofs, x1as = [], [], [], [], [], []
            for s in range(3):
                kt = constp.tile([128, GRP * KT_W], BF, name=f"kt{s}")
                kts.append(kt)
                an = constp.tile([128, GRP * AN_W], BF, name=f"an{s}")
                nc.vector.memset(an[0:64, :], 0.0)
                ans.append(an)
                qa = constp.tile([128, GRP * QA_W], BF, name=f"qa{s}")
                qas.append(qa)
                va = constp.tile([128, GRP * VA_W], BF, name=f"va{s}")
                vas.append(va)
                if s < 2:
                    of = constp.tile([128, GRP, 12, 32], BF, name=f"of{s}")
                    ofs.append(of)
                if s < 2:
                    # x1a [128, 3, 66]: per j: [f0 x(32) | f1 x(32) | den | den]
                    # rows 0-63 f0 (cols 0:32, den col 64), rows 64-127 f1
                    # (cols 32:64, den col 65); other cells stay zero.
                    x1a = constp.tile([128, 3, 66], BF, name=f"x1a{s}")
                    nc.vector.memset(x1a[:], 0.0)
                    nc.vector.memset(x1a[0:64, :, 64:65], 1.0)
                    nc.vector.memset(x1a[64:128, :, 65:66], 1.0)
                    x1as.append(x1a)

            win = []  # per-window live tiles

            # Each DMA path moves ~2.6 B/ns per partition-row and serializes
            # its transfers, so path time = sum(row_bytes)/2.6 per transfer.
            # Steady state per group: pool = kin + anc f0/f1 + vg (~4.8us),
            # sync = qin + out halves (~4.7us), within the ~6.3us group
            # budget; scalar carries the one-time constants.
            def dma_in_kq(g):
                s = g % 3
                nc.gpsimd.dma_start(kts[s][0:64, :], kin_d[g])
                anv = ans[s].rearrange("p (b c) -> p b c", c=128)
                nc.gpsimd.dma_start(
                    anv[0:32, :, 0:64], ain_d[g, 0].rearrange("p (b c) -> p b c", c=64))
                nc.gpsimd.dma_start(
                    anv[32:64, :, 64:128], ain_d[g, 1].rearrange("p (b c) -> p b c", c=64))
                nc.sync.dma_start(qas[s][0:64, :], qin_d[g])

            def dma_in_vg(g):
                nc.gpsimd.dma_start(vas[g % 3][:], vin_d[g])

            # Warmup: constants on the otherwise-idle scalar HWDGE ring,
            # interleaved per parity so group-0 compute starts after ~1
            # parity of consts (not all 3).
            for s in range(3):
                nc.scalar.dma_start(kts[s][64:128, :], kc_d[:])
                nc.scalar.dma_start(qas[s][64:128, :], qc_d[:])
                nc.scalar.dma_start(ans[s][64:128, :], vc_d[:])
                dma_in_kq(s)
                if s < 2:
                    dma_in_vg(s)

            for w in range(WPC + 3):
                if w < WPC:
                    g, r = w // GRP, w % GRP
                    s = g % 3
                    kt, qa, anb = kts[s], qas[s], ans[s]
                    k0, q0, a0 = r * KT_W, r * QA_W, r * AN_W
                    Sw = ps_s.tile([128, 1536], F32, tag="S")
                    # stage-1 QK head-paired + rank-32 bias rows: out
                    # (tok, anc f0|f1) per (j,t) block
                    for j in range(3):
                        for t in (0, 1):
                            b = 2 * j + t
                            nc.tensor.matmul(
                                Sw[:, 256 * j + 128 * t:256 * j + 128 * t + 128],
                                kt[:, k0 + 128 * b:k0 + 128 * b + 128],
                                anb[:, a0 + 128 * j:a0 + 128 * j + 128],
                                start=True, stop=True,
                            )
                    # stage-2 QK f-merged + rank-32 bias rows: out (anc-bd, tok)
                    for j in range(3):
                        nc.tensor.matmul(
                            Sw[:, S2_OFF + 256 * j:S2_OFF + 256 * j + 256],
                            qa[:, q0 + A2_OFF + 128 * j:q0 + A2_OFF + 128 * j + 128],
                            qa[:, q0 + 256 * j:q0 + 256 * j + 256],
                            start=True, stop=True,
                        )
                    ew = esp.tile([128, 1536], BF, tag="e")
                    nc.scalar.activation(ew[:], Sw[:], EXP)
                    win.append((ew, w))
                if w >= 3:
                    es2, xo2, xa2 = win[w - 3]
                    w2 = w - 3
                    # AV-2: out2 (tok, 66-block per s=3t+j), K=128 over anc-bd;
                    # writes into xo cols 99-495 after x1u reads complete
                    for t in (0, 1):
                        for j in range(3):
                            si = 3 * t + j
                            nc.tensor.matmul(
                                xo2[:, 99 + 66 * si:99 + 66 * si + 66],
                                es2[:, S2_OFF + 256 * j + 128 * t:S2_OFF + 256 * j + 128 * t + 128],
                                xa2[:, j, :],
                                start=True, stop=True,
                            )
                    ouv = xo2[:, 99:495].rearrange("p (m c) -> p m c", m=6)
                    ro = rop.tile([128, 6, 2, 1], F32, tag="ro")
                    nc.vector.reciprocal(
                        ro[:], ouv[:, :, 64:66].rearrange("p m (f c) -> p m f c", c=1))
                    ouv2 = ouv[:, :, 0:64].rearrange("p m (f c) -> p m f c", f=2)
                    of = ofs[(w2 // GRP) % 2]
                    ofv = of[:, w2 % GRP, :, :].rearrange("p (m f) c -> p m f c", m=6)
                    # split: blocks 0-1 (xo cols 99-231) first, releasing the
                    # xo WAR for the next AV-1 (writes cols 0-198) ~330ns
                    # earlier -- the PE stalls on this when DVE runs hot
                    nc.vector.tensor_tensor(
                        ofv[:, 0:2, :, :], ouv2[:, 0:2, :, :],
                        ro[:, 0:2, :, 0:1].to_broadcast((128, 2, 2, 32)), mybir.AluOpType.mult)
                    nc.vector.tensor_tensor(
                        ofv[:, 2:6, :, :], ouv2[:, 2:6, :, :],
                        ro[:, 2:6, :, 0:1].to_broadcast((128, 4, 2, 32)), mybir.AluOpType.mult)
                    # out flushed in half-group chunks; the final group goes
                    # per-window so the last DMA waits only on window 63
                    if w2 // GRP == NG - 1 and w2 % GRP >= 2:
                        nc.sync.dma_start(
                            out_d[w2 // GRP][:, w2 % GRP:w2 % GRP + 1],
                            of[:, w2 % GRP:w2 % GRP + 1])
                    elif w2 % GRP == 1:
                        nc.sync.dma_start(out_d[w2 // GRP][:, 0:2], of[:, 0:2])
                    elif w2 % GRP == GRP - 1 and w2 // GRP < NG - 1:
                        nc.sync.dma_start(out_d[w2 // GRP][:, 2:4], of[:, 2:4])
                    win[w2] = None
                if 2 <= w < WPC + 2:
                    es1, w1 = win[w - 2]
                    va1 = vas[(w1 // GRP) % 3]
                    v0 = (w1 % GRP) * VA_W
                    xo = ps_xo.tile([128, 512], F32, tag="xo")
                    # AV-1: x1u (anc-bd, 66-block per j), K=128 over tokens
                    for j in range(3):
                        for t in (0, 1):
                            nc.tensor.matmul(
                                xo[:, 66 * j:66 * j + 66],
                                es1[:, 256 * j + 128 * t:256 * j + 128 * t + 128],
                                va1[:, v0 + 198 * t + 66 * j:v0 + 198 * t + 66 * j + 66],
                                start=(t == 0), stop=(t == 1),
                            )
                    xov = xo[:, 0:198].rearrange("p (j c) -> p j c", j=3)
                    rd = rdp.tile([128, 3, 1], F32, tag="rd")
                    nc.vector.reciprocal(rd[:], xov[:, :, 64:65])
                    xa = x1as[(w - 2) % 2]
                    nc.vector.tensor_tensor(
                        xa[0:64, :, 0:32], xov[0:64, :, 0:32],
                        rd[0:64, :, 0:1].to_broadcast((64, 3, 32)), mybir.AluOpType.mult)
                    nc.vector.tensor_tensor(
                        xa[64:128, :, 32:64], xov[64:128, :, 32:64],
                        rd[64:128, :, 0:1].to_broadcast((64, 3, 32)), mybir.AluOpType.mult)
                    win[w - 2] = (es1, xo, xa)
                # prefetch group g+3 only after the lagged AV-1 of group g's
                # second window has been emitted, so its kt/qa reads
                # version-order before the overwrite of that parity
                if w >= 2 and (w - 2) % GRP == 1:
                    g_next = (w - 2) // GRP + 3
                    if g_next < NG:
                        dma_in_kq(g_next)
                if w >= 2 and (w - 2) % GRP == 3:
                    g_v = (w - 2) // GRP + 2
                    if 2 <= g_v < NG:
                        dma_in_vg(g_v)
    return nc


def _get_nc():
    if "nc" not in _CACHED:
        nc = _build_nc()
        nc.compile()
        _CACHED["nc"] = nc
    return _CACHED["nc"]


def _l2n(x):
    n = np.sqrt((x * x).sum(-1, keepdims=True))
    return x / np.maximum(n, 1e-12)


def _prepare(qkv, anchor, table, logit_scale1, cpb1_w1, cpb1_b1, cpb1_w2,
             logit_scale2, cpb2_w1, cpb2_b1, cpb2_w2, index_a2w, index_w2a):
    f32 = np.float32
    t2 = np.asarray(table, f32).reshape(-1, 2)
    bt1 = np.maximum(t2 @ np.asarray(cpb1_w1, f32) + np.asarray(cpb1_b1, f32), 0.0) @ np.asarray(cpb1_w2, f32)
    bt2 = np.maximum(t2 @ np.asarray(cpb2_w1, f32) + np.asarray(cpb2_b1, f32), 0.0) @ np.asarray(cpb2_w2, f32)
    ia = np.asarray(index_a2w).astype(np.int64).reshape(-1)
    iw = np.asarray(index_w2a).astype(np.int64).reshape(-1)
    b1 = (16.0 / (1.0 + np.exp(-bt1[ia]))).reshape(N2, N1, NUM_HEADS).transpose(2, 0, 1)
    b1c = b1 - b1.mean(axis=2, keepdims=True)      # center over tokens (softmax axis)
    b2 = (16.0 / (1.0 + np.exp(-bt2[iw]))).reshape(N1, N2, NUM_HEADS).transpose(2, 0, 1)
    b2c = b2 - b2.mean(axis=2, keepdims=True)      # center over anchors (softmax axis)
    u2 = np.zeros((NUM_HEADS, N1, RANK2), f32)
    v2 = np.zeros((NUM_HEADS, N2, RANK2), f32)
    w1 = np.zeros((NUM_HEADS, RANK2, N1), f32)     # stage-1 factors over tokens
    v1 = np.zeros((NUM_HEADS, RANK2, N2), f32)     # stage-1 factors over anchors
    for h in range(NUM_HEADS):
        U, Sv, Vt = np.linalg.svd(b2c[h], full_matrices=False)
        u2[h] = U[:, :RANK2] * np.sqrt(Sv[:RANK2])
        v2[h] = (Vt[:RANK2] * np.sqrt(Sv[:RANK2])[:, None]).T
        U, Sv, Vt = np.linalg.svd(b1c[h], full_matrices=False)   # (anc, tok)
        v1[h] = (U[:, :RANK2] * np.sqrt(Sv[:RANK2])).T
        w1[h] = Vt[:RANK2] * np.sqrt(Sv[:RANK2])[:, None]

    s1 = np.exp(np.minimum(np.asarray(logit_scale1, f32).reshape(NUM_HEADS), LOGIT_MAX))
    s2 = np.exp(np.minimum(np.asarray(logit_scale2, f32).reshape(NUM_HEADS), LOGIT_MAX))

    qkv4 = np.ascontiguousarray(np.asarray(qkv, f32).reshape(B, 16, STRIPE, 16, STRIPE, 3 * DIM)
                                .transpose(0, 1, 3, 2, 4, 5)).reshape(NWIN, N1, 3 * DIM)
    q = qkv4[:, :, :DIM].reshape(NWIN, N1, NUM_HEADS, HD)
    k = qkv4[:, :, DIM:2 * DIM].reshape(NWIN, N1, NUM_HEADS, HD)
    v = qkv4[:, :, 2 * DIM:].reshape(NWIN, N1, NUM_HEADS, HD)
    anc4 = np.ascontiguousarray(np.asarray(anchor, f32).reshape(B, 16, ANCH, 16, ANCH, DIM)
                                .transpose(0, 1, 3, 2, 4, 5)).reshape(NWIN, N2, NUM_HEADS, HD)

    kn = _l2n(k) * s1[None, None, :, None]
    qn = _l2n(q) * s2[None, None, :, None]
    an = _l2n(anc4)

    kin = np.zeros((NWIN, 64, KT_W), BF16)
    for j in range(3):
        for t in (0, 1):
            b = 2 * j + t
            kin[:, 0:32, 128 * b:128 * b + 128] = kn[:, 128 * t:128 * t + 128, j, :].transpose(0, 2, 1)
            kin[:, 32:64, 128 * b:128 * b + 128] = kn[:, 128 * t:128 * t + 128, j + 3, :].transpose(0, 2, 1)

    ain = np.zeros((NWIN, 2, 32, 192), BF16)
    for j in range(3):
        for f in (0, 1):
            ain[:, f, :, 64 * j:64 * j + 64] = an[:, :, j + 3 * f, :].transpose(0, 2, 1)

    kc = np.zeros((64, KT_W), f32)
    for j in range(3):
        for t in (0, 1):
            b = 2 * j + t
            kc[0:32, 128 * b:128 * b + 128] = w1[j][:, 128 * t:128 * t + 128]
            kc[32:64, 128 * b:128 * b + 128] = w1[j + 3][:, 128 * t:128 * t + 128]

    vc = np.zeros((64, AN_W), f32)
    for j in range(3):
        vc[0:32, 128 * j:128 * j + 64] = v1[j]
        vc[32:64, 128 * j + 64:128 * j + 128] = v1[j + 3]

    qin = np.zeros((NWIN, 64, QA_W), BF16)
    qc = np.zeros((64, QA_W), f32)
    for j in range(3):
        qin[:, 0:32, 256 * j:256 * j + 256] = qn[:, :, j, :].transpose(0, 2, 1)
        qin[:, 32:64, 256 * j:256 * j + 256] = qn[:, :, j + 3, :].transpose(0, 2, 1)
        qin[:, 0:32, A2_OFF + 128 * j:A2_OFF + 128 * j + 64] = an[:, :, j, :].transpose(0, 2, 1)
        qin[:, 32:64, A2_OFF + 128 * j + 64:A2_OFF + 128 * j + 128] = an[:, :, j + 3, :].transpose(0, 2, 1)
        qc[0:32, 256 * j:256 * j + 256] = u2[j].T
        qc[32:64, 256 * j:256 * j + 256] = u2[j + 3].T
        qc[0:32, A2_OFF + 128 * j:A2_OFF + 128 * j + 64] = v2[j].T
        qc[32:64, A2_OFF + 128 * j + 64:A2_OFF + 128 * j + 128] = v2[j + 3].T

    vin = np.zeros((NWIN, 128, VA_W), BF16)
    for t in (0, 1):
        for j in range(3):
            c0 = 198 * t + 66 * j
            vin[:, :, c0:c0 + 32] = v[:, 128 * t:128 * (t + 1), j, :]
            vin[:, :, c0 + 32:c0 + 64] = v[:, 128 * t:128 * (t + 1), j + 3, :]
            vin[:, :, c0 + 64] = 1.0

    GRP = 4
    NG = WPC // GRP
    kc_bf = np.tile(kc.astype(BF16), (1, GRP))
    qc_bf = np.tile(qc.astype(BF16), (1, GRP))
    vc_bf = np.tile(vc.astype(BF16), (1, GRP))

    def grp(x, width):
        # (WPC, P, W) -> (NG, P, GRP*W) with windows side by side
        xc = x.reshape(NG, GRP, x.shape[1], width)
        return np.ascontiguousarray(xc.transpose(0, 2, 1, 3)).reshape(NG, x.shape[1], GRP * width)

    in_maps = []
    for c in range(NCORES):
        asl = ain[c * WPC:(c + 1) * WPC].reshape(NG, GRP, 2, 32, 192)
        in_maps.append({
            "kin": grp(kin[c * WPC:(c + 1) * WPC], KT_W),
            "ain": np.ascontiguousarray(asl.transpose(0, 2, 3, 1, 4)).reshape(NG, 2, 32, GRP * 192),
            "qin": grp(qin[c * WPC:(c + 1) * WPC], QA_W),
            "vin": grp(vin[c * WPC:(c + 1) * WPC], VA_W),
            "kc": kc_bf,
            "vc": vc_bf,
            "qc": qc_bf,
        })
    return in_maps


def _assemble(results):
    outw = np.concatenate(
        [np.asarray(r["out"], np.float32).transpose(0, 2, 1, 3, 4)
         .reshape(WPC, 128, 2, 3, 2, 32) for r in results],
        axis=0)  # (512, p, t, j, f, i); head h = j + 3f, token = 128t + p
    full = outw.transpose(0, 2, 1, 4, 3, 5).reshape(NWIN, N1, DIM)
    img = full.reshape(B, 16, 16, STRIPE, STRIPE, DIM).transpose(0, 1, 3, 2, 4, 5)
    return np.ascontiguousarray(img).reshape(B, HS * HS, DIM)


def _run(inputs, trace=False, trace_kwargs=None):
    in_maps = _prepare(
        inputs["qkv"], inputs["anchor"], inputs["table"],
        inputs["logit_scale1"], inputs["cpb1_w1"], inputs["cpb1_b1"], inputs["cpb1_w2"],
        inputs["logit_scale2"], inputs["cpb2_w1"], inputs["cpb2_b1"], inputs["cpb2_w2"],
        inputs["index_a2w"], inputs["index_w2a"],
    )
    nc = _get_nc()
    res = run_bass_kernel_spmd(
        nc, in_maps, core_ids=list(range(NCORES)),
        trace=trace, **(trace_kwargs or {}),
    )
    out = _assemble(res.results)
    return out, res


def kernel(**inputs):
    out, _ = _run(inputs, trace=False)
    return out



# revision 39
# speedup vs baseline: 1.1442x; 1.1442x over previous
"""AnchorStripeAttention Trainium2 kernel (8 NeuronCores, data-parallel over windows).

Host: window-partition + per-head l2norm + logit-scale fold. CPB bias folded
into the QK matmuls as extra contraction rows, rank-32 factorized for BOTH
stages (bias centered along the softmax axis first; the dropped mean is
softmax-invariant). Heads are paired (j, j+3) block-diagonally so every QK
matmul is a full [K=128, M=128] stationary.

Device (per window): 6 stage-1 QK matmuls (K=128: k 2x32 + W1 2x32, N=128:
anc|V1 block-diag), 3 stage-2 QK matmuls (K=128, N=256), ONE merged exp over
the [128,1536] score tile on ACT, 12 AV matmuls (K=128, N=66) with
ones-column denominators, normalize on DVE.

Scheduling notes:
- Both stages' scores live in ONE [128,1536] PSUM tile (3 banks, 2-slot
  rotation + 2 xo banks = 8 banks exactly). One exp per window saves ~240ns
  of ACT overhead/window; ACT (~1.54us/window) is the steady-state pacer.
- DMA: each path (sync HWDGE / scalar HWDGE / pool SWDGE) moves ~2.6 B/ns
  per partition-row and serializes its transfers, so path time =
  sum(row_bytes)/2.6. Per 4-window group: pool = kin [64-row] + anc f0/f1
  strided [32-row] + vin [128-row] (~4.8us), sync = qin [64-row] + out
  halves [128-row] (~4.7us) vs a ~6.3us group budget. Constants stream on
  scalar, interleaved per parity so group 0 starts after one parity's worth.
- 3 parity sets of kt/an/qa/va tiles (const rows live below the DMA rows);
  kin/qin prefetch 3 groups ahead, vin 2 groups, out flushed in half-groups.
- The rank-32 stage-1 bias costs ~5e-3 extra rel err (1.4e-2 total vs 2e-2
  budget) and makes kin a 64-row DMA (2x faster path class), kills the I64
  identity columns, and shrinks the replicated constants ~40%.
"""

import math
import sys

import numpy as np

if "/opt/trn_rl_repo" not in sys.path:
    sys.path.insert(0, "/opt/trn_rl_repo")

import concourse.bass as bass  # noqa: E402
import concourse.bacc as bacc  # noqa: E402
import concourse.tile as tile  # noqa: E402
from concourse import mybir  # noqa: E402
from concourse.bass_utils import run_bass_kernel_spmd  # noqa: E402

import ml_dtypes  # noqa: E402

BF16 = np.dtype(ml_dtypes.bfloat16)

NUM_HEADS = 6
DIM = 192
HD = 32
STRIPE = 16
ANCH = 8
B = 2
HS = 256
N1 = STRIPE * STRIPE  # 256 window tokens
N2 = ANCH * ANCH      # 64 anchor tokens
NWIN = 512
NCORES = 8
WPC = NWIN // NCORES  # 64 windows per core
LOGIT_MAX = math.log(1.0 / 0.01)
RANK2 = 32            # stage-2 bias factor rank

# kt tile [128, 768]: block (j,t) at cols 128*(2j+t): rows 0:32 k_j^T,
# rows 32:64 k_{j+3}^T (per-window DMA); rows 64:96 W1_j, 96:128 W1_{j+3}
# (const rank-32 stage-1 bias factors over tokens).
KT_W = 768
# an tile [128, 384]: block j at cols 128j; cols 0:64 (f0): rows 0:32
# anc_j^T, rows 64:96 V1_j; cols 64:128 (f1): rows 32:64 anc_{j+3}^T,
# rows 96:128 V1_{j+3}; other cells zero (block-diag head pairing).
AN_W = 384
# qa tile [128, 1152]: rows 0-63 per-window; cols 256j hold qT stacked f0/f1;
# cols 768+128j hold anc block-diag. rows 64-127: u2 stacked + v2 bd (const).
QA_W = 1152
A2_OFF = 768
# va tile [128, 396]: block (t,j) at 198t+66j: [f0 v(32) | f1 v(32) | one | 0]
VA_W = 396
S_W = 1536            # psum: stage1 (j,t,f) 64-blocks at 256j+128t+64f; stage2 at 768+256j
S2_OFF = 768          # stage-2 score offset within the merged [128,1536] S tile

_CACHED = {}


def _build_nc():
    BF = mybir.dt.bfloat16
    F32 = mybir.dt.float32
    EXP = mybir.ActivationFunctionType.Exp

    GRP = 4                   # windows per DMA group
    NG = WPC // GRP

    nc = bacc.Bacc(None)
    kin_d = nc.dram_tensor("kin", [NG, 64, GRP * KT_W], BF, kind="ExternalInput")
    ain_d = nc.dram_tensor("ain", [NG, 2, 32, GRP * 192], BF, kind="ExternalInput")
    qin_d = nc.dram_tensor("qin", [NG, 64, GRP * QA_W], BF, kind="ExternalInput")
    vin_d = nc.dram_tensor("vin", [NG, 128, GRP * VA_W], BF, kind="ExternalInput")
    kc_d = nc.dram_tensor("kc", [64, GRP * KT_W], BF, kind="ExternalInput")
    vc_d = nc.dram_tensor("vc", [64, GRP * AN_W], BF, kind="ExternalInput")
    qc_d = nc.dram_tensor("qc", [64, GRP * QA_W], BF, kind="ExternalInput")
    out_d = nc.dram_tensor("out", [NG, 128, GRP, 12, 32], BF, kind="ExternalOutput")

    with tile.TileContext(nc) as tc:
        with (
            tc.tile_pool(name="const", bufs=1) as constp,
            tc.tile_pool(name="esp", bufs=5) as esp,
            tc.tile_pool(name="rdp", bufs=2) as rdp,
            tc.tile_pool(name="rop", bufs=2) as rop,
            tc.tile_pool(name="ps_s", bufs=2, space="PSUM") as ps_s,
            tc.tile_pool(name="ps_xo", bufs=2, space="PSUM") as ps_xo,
        ):
            kts, ans, qas, vas, ofs, x1as = [], [], [], [], [], []
            for s in range(3):
                kt = constp.tile([128, GRP * KT_W], BF, name=f"kt{s}")
                kts.append(kt)
                an = constp.tile([128, GRP * AN_W], BF, name=f"an{s}")
                nc.vector.memset(an[0:64, :], 0.0)
                ans.append(an)
                qa = constp.tile([128, GRP * QA_W], BF, name=f"qa{s}")
                qas.append(qa)
                va = constp.tile([128, GRP * VA_W], BF, name=f"va{s}")
                vas.append(va)
                if s < 2:
                    of = constp.tile([128, GRP, 12, 32], BF, name=f"of{s}")
                    ofs.append(of)
                if s < 2:
                    # x1a [128, 3, 66]: per j: [f0 x(32) | f1 x(32) | den | den]
                    # rows 0-63 f0 (cols 0:32, den col 64), rows 64-127 f1
                    # (cols 32:64, den col 65); other cells stay zero.
                    x1a = constp.tile([128, 3, 66], BF, name=f"x1a{s}")
                    nc.vector.memset(x1a[:], 0.0)
                    nc.vector.memset(x1a[0:64, :, 64:65], 1.0)
                    nc.vector.memset(x1a[64:128, :, 65:66], 1.0)
                    x1as.append(x1a)

            win = []  # per-window live tiles

            # Each DMA path moves ~2.6 B/ns per partition-row and serializes
            # its transfers, so path time = sum(row_bytes)/2.6 per transfer.
            # Steady state per group: pool = kin + anc f0/f1 + vg (~4.8us),
            # sync = qin + out halves (~4.7us), within the ~6.3us group
            # budget; scalar carries the one-time constants.
            def dma_in_kq(g):
                s = g % 3
                nc.gpsimd.dma_start(kts[s][0:64, :], kin_d[g])
                anv = ans[s].rearrange("p (b c) -> p b c", c=128)
                nc.gpsimd.dma_start(
                    anv[0:32, :, 0:64], ain_d[g, 0].rearrange("p (b c) -> p b c", c=64))
                nc.gpsimd.dma_start(
                    anv[32:64, :, 64:128], ain_d[g, 1].rearrange("p (b c) -> p b c", c=64))
                nc.sync.dma_start(qas[s][0:64, :], qin_d[g])

            def dma_in_vg(g):
                nc.gpsimd.dma_start(vas[g % 3][:], vin_d[g])

            # Warmup: constants on the otherwise-idle scalar HWDGE ring,
            # interleaved per parity so group-0 compute starts after ~1
            # parity of consts (not all 3).
            for s in range(3):
                nc.scalar.dma_start(kts[s][64:128, :], kc_d[:])
                nc.scalar.dma_start(qas[s][64:128, :], qc_d[:])
                nc.scalar.dma_start(ans[s][64:128, :], vc_d[:])
                dma_in_kq(s)
                if s < 2:
                    dma_in_vg(s)

            for w in range(WPC + 3):
                if w < WPC:
                    g, r = w // GRP, w % GRP
                    s = g % 3
                    kt, qa, anb = kts[s], qas[s], ans[s]
                    k0, q0, a0 = r * KT_W, r * QA_W, r * AN_W
                    Sw = ps_s.tile([128, 1536], F32, tag="S")
                    # stage-1 QK head-paired + rank-32 bias rows: out
                    # (tok, anc f0|f1) per (j,t) block
                    for j in range(3):
                        for t in (0, 1):
                            b = 2 * j + t
                            nc.tensor.matmul(
                                Sw[:, 256 * j + 128 * t:256 * j + 128 * t + 128],
                                kt[:, k0 + 128 * b:k0 + 128 * b + 128],
                                anb[:, a0 + 128 * j:a0 + 128 * j + 128],
                                start=True, stop=True,
                            )
                    # stage-2 QK f-merged + rank-32 bias rows: out (anc-bd, tok)
                    for j in range(3):
                        nc.tensor.matmul(
                            Sw[:, S2_OFF + 256 * j:S2_OFF + 256 * j + 256],
                            qa[:, q0 + A2_OFF + 128 * j:q0 + A2_OFF + 128 * j + 128],
                            qa[:, q0 + 256 * j:q0 + 256 * j + 256],
                            start=True, stop=True,
                        )
                    ew = esp.tile([128, 1536], BF, tag="e")
                    nc.scalar.activation(ew[:], Sw[:], EXP)
                    win.append((ew, w))
                if w >= 3:
                    es2, xo2, xa2 = win[w - 3]
                    w2 = w - 3
                    # AV-2: out2 (tok, 66-block per s=3t+j), K=128 over anc-bd;
                    # writes into xo cols 99-495 after x1u reads complete
                    for t in (0, 1):
                        for j in range(3):
                            si = 3 * t + j
                            nc.tensor.matmul(
                                xo2[:, 99 + 66 * si:99 + 66 * si + 66],
                                es2[:, S2_OFF + 256 * j + 128 * t:S2_OFF + 256 * j + 128 * t + 128],
                                xa2[:, j, :],
                                start=True, stop=True,
                            )
                    ouv = xo2[:, 99:495].rearrange("p (m c) -> p m c", m=6)
                    ro = rop.tile([128, 6, 2, 1], F32, tag="ro")
                    nc.vector.reciprocal(
                        ro[:], ouv[:, :, 64:66].rearrange("p m (f c) -> p m f c", c=1))
                    ouv2 = ouv[:, :, 0:64].rearrange("p m (f c) -> p m f c", f=2)
                    of = ofs[(w2 // GRP) % 2]
                    ofv = of[:, w2 % GRP, :, :].rearrange("p (m f) c -> p m f c", m=6)
                    nc.vector.tensor_tensor(
                        ofv[:], ouv2[:],
                        ro[:, :, :, 0:1].to_broadcast((128, 6, 2, 32)), mybir.AluOpType.mult)
                    # out flushed in half-group chunks; the final group goes
                    # per-window so the last DMA waits only on window 63
                    if w2 // GRP == NG - 1 and w2 % GRP >= 2:
                        nc.sync.dma_start(
                            out_d[w2 // GRP][:, w2 % GRP:w2 % GRP + 1],
                            of[:, w2 % GRP:w2 % GRP + 1])
                    elif w2 % GRP == 1:
                        nc.sync.dma_start(out_d[w2 // GRP][:, 0:2], of[:, 0:2])
                    elif w2 % GRP == GRP - 1 and w2 // GRP < NG - 1:
                        nc.sync.dma_start(out_d[w2 // GRP][:, 2:4], of[:, 2:4])
                    win[w2] = None
                if 2 <= w < WPC + 2:
                    es1, w1 = win[w - 2]
                    va1 = vas[(w1 // GRP) % 3]
                    v0 = (w1 % GRP) * VA_W
                    xo = ps_xo.tile([128, 512], F32, tag="xo")
                    # AV-1: x1u (anc-bd, 66-block per j), K=128 over tokens
                    for j in range(3):
                        for t in (0, 1):
                            nc.tensor.matmul(
                                xo[:, 66 * j:66 * j + 66],
                                es1[:, 256 * j + 128 * t:256 * j + 128 * t + 128],
                                va1[:, v0 + 198 * t + 66 * j:v0 + 198 * t + 66 * j + 66],
                                start=(t == 0), stop=(t == 1),
                            )
                    xov = xo[:, 0:198].rearrange("p (j c) -> p j c", j=3)
                    rd = rdp.tile([128, 3, 1], F32, tag="rd")
                    nc.vector.reciprocal(rd[:], xov[:, :, 64:65])
                    xa = x1as[(w - 2) % 2]
                    nc.vector.tensor_tensor(
                        xa[0:64, :, 0:32], xov[0:64, :, 0:32],
                        rd[0:64, :, 0:1].to_broadcast((64, 3, 32)), mybir.AluOpType.mult)
                    nc.vector.tensor_tensor(
                        xa[64:128, :, 32:64], xov[64:128, :, 32:64],
                        rd[64:128, :, 0:1].to_broadcast((64, 3, 32)), mybir.AluOpType.mult)
                    win[w - 2] = (es1, xo, xa)
                # prefetch group g+3 only after the lagged AV-1 of group g's
                # second window has been emitted, so its kt/qa reads
                # version-order before the overwrite of that parity
                if w >= 2 and (w - 2) % GRP == 1:
                    g_next = (w - 2) // GRP + 3
                    if g_next < NG:
                        dma_in_kq(g_next)
                if w >= 2 and (w - 2) % GRP == 3:
                    g_v = (w - 2) // GRP + 2
                    if 2 <= g_v < NG:
                        dma_in_vg(g_v)
    return nc


def _get_nc():
    if "nc" not in _CACHED:
        nc = _build_nc()
        nc.compile()
        _CACHED["nc"] = nc
    return _CACHED["nc"]


def _l2n(x):
    n = np.sqrt((x * x).sum(-1, keepdims=True))
    return x / np.maximum(n, 1e-12)


def _prepare(qkv, anchor, table, logit_scale1, cpb1_w1, cpb1_b1, cpb1_w2,
             logit_scale2, cpb2_w1, cpb2_b1, cpb2_w2, index_a2w, index_w2a):
    f32 = np.float32
    t2 = np.asarray(table, f32).reshape(-1, 2)
    bt1 = np.maximum(t2 @ np.asarray(cpb1_w1, f32) + np.asarray(cpb1_b1, f32), 0.0) @ np.asarray(cpb1_w2, f32)
    bt2 = np.maximum(t2 @ np.asarray(cpb2_w1, f32) + np.asarray(cpb2_b1, f32), 0.0) @ np.asarray(cpb2_w2, f32)
    ia = np.asarray(index_a2w).astype(np.int64).reshape(-1)
    iw = np.asarray(index_w2a).astype(np.int64).reshape(-1)
    b1 = (16.0 / (1.0 + np.exp(-bt1[ia]))).reshape(N2, N1, NUM_HEADS).transpose(2, 0, 1)
    b1c = b1 - b1.mean(axis=2, keepdims=True)      # center over tokens (softmax axis)
    b2 = (16.0 / (1.0 + np.exp(-bt2[iw]))).reshape(N1, N2, NUM_HEADS).transpose(2, 0, 1)
    b2c = b2 - b2.mean(axis=2, keepdims=True)      # center over anchors (softmax axis)
    u2 = np.zeros((NUM_HEADS, N1, RANK2), f32)
    v2 = np.zeros((NUM_HEADS, N2, RANK2), f32)
    w1 = np.zeros((NUM_HEADS, RANK2, N1), f32)     # stage-1 factors over tokens
    v1 = np.zeros((NUM_HEADS, RANK2, N2), f32)     # stage-1 factors over anchors
    for h in range(NUM_HEADS):
        U, Sv, Vt = np.linalg.svd(b2c[h], full_matrices=False)
        u2[h] = U[:, :RANK2] * np.sqrt(Sv[:RANK2])
        v2[h] = (Vt[:RANK2] * np.sqrt(Sv[:RANK2])[:, None]).T
        U, Sv, Vt = np.linalg.svd(b1c[h], full_matrices=False)   # (anc, tok)
        v1[h] = (U[:, :RANK2] * np.sqrt(Sv[:RANK2])).T
        w1[h] = Vt[:RANK2] * np.sqrt(Sv[:RANK2])[:, None]

    s1 = np.exp(np.minimum(np.asarray(logit_scale1, f32).reshape(NUM_HEADS), LOGIT_MAX))
    s2 = np.exp(np.minimum(np.asarray(logit_scale2, f32).reshape(NUM_HEADS), LOGIT_MAX))

    qkv4 = np.ascontiguousarray(np.asarray(qkv, f32).reshape(B, 16, STRIPE, 16, STRIPE, 3 * DIM)
                                .transpose(0, 1, 3, 2, 4, 5)).reshape(NWIN, N1, 3 * DIM)
    q = qkv4[:, :, :DIM].reshape(NWIN, N1, NUM_HEADS, HD)
    k = qkv4[:, :, DIM:2 * DIM].reshape(NWIN, N1, NUM_HEADS, HD)
    v = qkv4[:, :, 2 * DIM:].reshape(NWIN, N1, NUM_HEADS, HD)
    anc4 = np.ascontiguousarray(np.asarray(anchor, f32).reshape(B, 16, ANCH, 16, ANCH, DIM)
                                .transpose(0, 1, 3, 2, 4, 5)).reshape(NWIN, N2, NUM_HEADS, HD)

    kn = _l2n(k) * s1[None, None, :, None]
    qn = _l2n(q) * s2[None, None, :, None]
    an = _l2n(anc4)

    kin = np.zeros((NWIN, 64, KT_W), BF16)
    for j in range(3):
        for t in (0, 1):
            b = 2 * j + t
            kin[:, 0:32, 128 * b:128 * b + 128] = kn[:, 128 * t:128 * t + 128, j, :].transpose(0, 2, 1)
            kin[:, 32:64, 128 * b:128 * b + 128] = kn[:, 128 * t:128 * t + 128, j + 3, :].transpose(0, 2, 1)

    ain = np.zeros((NWIN, 2, 32, 192), BF16)
    for j in range(3):
        for f in (0, 1):
            ain[:, f, :, 64 * j:64 * j + 64] = an[:, :, j + 3 * f, :].transpose(0, 2, 1)

    kc = np.zeros((64, KT_W), f32)
    for j in range(3):
        for t in (0, 1):
            b = 2 * j + t
            kc[0:32, 128 * b:128 * b + 128] = w1[j][:, 128 * t:128 * t + 128]
            kc[32:64, 128 * b:128 * b + 128] = w1[j + 3][:, 128 * t:128 * t + 128]

    vc = np.zeros((64, AN_W), f32)
    for j in range(3):
        vc[0:32, 128 * j:128 * j + 64] = v1[j]
        vc[32:64, 128 * j + 64:128 * j + 128] = v1[j + 3]

    qin = np.zeros((NWIN, 64, QA_W), BF16)
    qc = np.zeros((64, QA_W), f32)
    for j in range(3):
        qin[:, 0:32, 256 * j:256 * j + 256] = qn[:, :, j, :].transpose(0, 2, 1)
        qin[:, 32:64, 256 * j:256 * j + 256] = qn[:, :, j + 3, :].transpose(0, 2, 1)
        qin[:, 0:32, A2_OFF + 128 * j:A2_OFF + 128 * j + 64] = an[:, :, j, :].transpose(0, 2, 1)
        qin[:, 32:64, A2_OFF + 128 * j + 64:A2_OFF + 128 * j + 128] = an[:, :, j + 3, :].transpose(0, 2, 1)
        qc[0:32, 256 * j:256 * j + 256] = u2[j].T
        qc[32:64, 256 * j:256 * j + 256] = u2[j + 3].T
        qc[0:32, A2_OFF + 128 * j:A2_OFF + 128 * j + 64] = v2[j].T
        qc[32:64, A2_OFF + 128 * j + 64:A2_OFF + 128 * j + 128] = v2[j + 3].T

    vin = np.zeros((NWIN, 128, VA_W), BF16)
    for t in (0, 1):
        for j in range(3):
            c0 = 198 * t + 66 * j
            vin[:, :, c0:c0 + 32] = v[:, 128 * t:128 * (t + 1), j, :]
            vin[:, :, c0 + 32:c0 + 64] = v[:, 128 * t:128 * (t + 1), j + 3, :]
            vin[:, :, c0 + 64] = 1.0

    GRP = 4
    NG = WPC // GRP
    kc_bf = np.tile(kc.astype(BF16), (1, GRP))
    qc_bf = np.tile(qc.astype(BF16), (1, GRP))
    vc_bf = np.tile(vc.astype(BF16), (1, GRP))

    def grp(x, width):
        # (WPC, P, W) -> (NG, P, GRP*W) with windows side by side
        xc = x.reshape(NG, GRP, x.shape[1], width)
        return np.ascontiguousarray(xc.transpose(0, 2, 1, 3)).reshape(NG, x.shape[1], GRP * width)

    in_maps = []
    for c in range(NCORES):
        asl = ain[c * WPC:(c + 1) * WPC].reshape(NG, GRP, 2, 32, 192)
        in_maps.append({
            "kin": grp(kin[c * WPC:(c + 1) * WPC], KT_W),
            "ain": np.ascontiguousarray(asl.transpose(0, 2, 3, 1, 4)).reshape(NG, 2, 32, GRP * 192),
            "qin": grp(qin[c * WPC:(c + 1) * WPC], QA_W),
            "vin": grp(vin[c * WPC:(c + 1) * WPC], VA_W),
            "kc": kc_bf,
            "vc": vc_bf,
            "qc": qc_bf,
        })
    return in_maps


def _assemble(results):
    outw = np.concatenate(
        [np.asarray(r["out"], np.float32).transpose(0, 2, 1, 3, 4)
         .reshape(WPC, 128, 2, 3, 2, 32) for r in results],
        axis=0)  # (512, p, t, j, f, i); head h = j + 3f, token = 128t + p
    full = outw.transpose(0, 2, 1, 4, 3, 5).reshape(NWIN, N1, DIM)
    img = full.reshape(B, 16, 16, STRIPE, STRIPE, DIM).transpose(0, 1, 3, 2, 4, 5)
    return np.ascontiguousarray(img).reshape(B, HS * HS, DIM)


def _run(inputs, trace=False, trace_kwargs=None):
    in_maps = _prepare(
        inputs["qkv"], inputs["anchor"], inputs["table"],
        inputs["logit_scale1"], inputs["cpb1_w1"], inputs["cpb1_b1"], inputs["cpb1_w2"],
        inputs["logit_scale2"], inputs["cpb2_w1"], inputs["cpb2_b1"], inputs["cpb2_w2"],
        inputs["index_a2w"], inputs["index_w2a"],
    )
    nc = _get_nc()
    res = run_bass_kernel_spmd(
        nc, in_maps, core_ids=list(range(NCORES)),
        trace=trace, **(trace_kwargs or {}),
    )
    out = _assemble(res.results)
    return out, res


def kernel(**inputs):
    out, _ = _run(inputs, trace=False)
    return out

